# revision 5
# baseline (speedup 1.0000x reference)
"""TRN2 Bass kernel for nn_BERTse_76227079569428 (6-layer BERT-style encoder).

Sharding: 8 cores = batch(4) x token-half(2). Core 2b+s owns tokens
[s*512,(s+1)*512) of batch b. All phases are token-parallel except attention
K/V, exchanged within (2b, 2b+1) pairs via per-layer AllGathers.

On-chip layout: feature-major residual stream x^T [D=1024(8 ptiles), L=512]
f32; matmul operands f16; scores transposed S^T[j,i]; softmax rowsums fused
into the attn@V matmul via an appended ones-column on V; LN stats via
ones-matmuls on the PE; per-token broadcasts via gpsimd partition_broadcast.
"""
import sys
import numpy as np

sys.path.insert(0, "/opt/trn_rl_repo")

import concourse.bass as bass
import concourse.mybir as mybir
import concourse.tile as tile
from concourse import bacc
from concourse.bass_utils import run_bass_kernel_spmd

F32 = mybir.dt.float32
F16 = mybir.dt.float16
AF = mybir.ActivationFunctionType
OP = mybir.AluOpType

D = 1024
DFF = 4096
NL = 6
H = 16
NC_HEADS = 8
T = 1024
L = 512          # local tokens per core
B = 4
DK = 64
LAMBD = 5.0
EPS = 1e-12
NT = D // 128    # 8 feature tiles
LT = L // 128    # 4 local token tiles
GROUPS = [[0, 1], [2, 3], [4, 5], [6, 7]]

LN_POS, LN_SEG = 0, 1
def LN_IN(l): return 2 + 2 * l
def LN_OUT(l): return 3 + 2 * l
LN_FINAL = 14
N_LN = 15


def build(n_layers=NL, use_highway=True):
    nc = bacc.Bacc("TRN2", target_bir_lowering=False)

    # ---------------- DRAM parameters ----------------
    x0t = nc.declare_dram_parameter("x0t", [D, L], F32, isOutput=False)
    post = nc.declare_dram_parameter("post", [D, L], F32, isOutput=False)
    segt = nc.declare_dram_parameter("segt", [D, L], F32, isOutput=False)
    cb16 = nc.declare_dram_parameter("cb16", [NC_HEADS, T, L], F16, isOutput=False)
    whw = nc.declare_dram_parameter("whw", [5, D, D], F16, isOutput=False)
    bhw = nc.declare_dram_parameter("bhw", [5, 128, NT], F32, isOutput=False)
    wq = nc.declare_dram_parameter("wq", [NL, D, D], F16, isOutput=False)
    wk = nc.declare_dram_parameter("wk", [NL, D, D], F16, isOutput=False)
    wv = nc.declare_dram_parameter("wv", [NL, D, D], F16, isOutput=False)
    wo = nc.declare_dram_parameter("wo", [NL, D, D], F16, isOutput=False)
    wf1 = nc.declare_dram_parameter("wf1", [NL, D, DFF], F16, isOutput=False)
    wf2 = nc.declare_dram_parameter("wf2", [NL, DFF, D], F16, isOutput=False)
    bqko = nc.declare_dram_parameter("bqko", [NL, 3, 128, NT], F32, isOutput=False)
    bvr = nc.declare_dram_parameter("bvr", [NL, D], F16, isOutput=False)
    bf1 = nc.declare_dram_parameter("bf1", [NL, 128, DFF // 128], F32, isOutput=False)
    bf2 = nc.declare_dram_parameter("bf2", [NL, 128, NT], F32, isOutput=False)
    ln_wb = nc.declare_dram_parameter("ln_wb", [N_LN, 2, 128, NT], F32, isOutput=False)
    wcls = nc.declare_dram_parameter("wcls", [D, 3], F16, isOutput=False)
    bclsb = nc.declare_dram_parameter("bclsb", [128, 3], F32, isOutput=False)
    out = nc.declare_dram_parameter("out", [L, 3], F32, isOutput=True)

    with tile.TileContext(nc) as tc:
        from contextlib import ExitStack
        es = ExitStack()
        sb = es.enter_context(tc.tile_pool(name="sb", bufs=1))
        ps = es.enter_context(tc.tile_pool(name="ps", bufs=1, space="PSUM"))
        dram = es.enter_context(tc.tile_pool(name="dram", bufs=1, space="DRAM"))

        # ---------------- constants & params in SBUF ----------------
        ones_col = sb.tile([128, 1], F16)          # stats lhsT  (K=128, M=1)
        nc.vector.memset(ones_col[:], 1.0)
        ones_row = sb.tile([1, 128], F16)          # bias-add lhsT (K=1, M=128)
        nc.vector.memset(ones_row[:], 1.0)

        lnp = sb.tile([128, N_LN, 2, NT], F32)
        nc.sync.dma_start(out=lnp[:], in_=ln_wb[:].rearrange("l c p t -> p l c t"))
        bqko_t = sb.tile([128, NL, 3, NT], F32)
        nc.sync.dma_start(out=bqko_t[:], in_=bqko[:].rearrange("l c p t -> p l c t"))
        bf1_t = sb.tile([128, NL, DFF // 128], F32)
        nc.sync.dma_start(out=bf1_t[:], in_=bf1[:].rearrange("l p t -> p l t"))
        bf2_t = sb.tile([128, NL, NT], F32)
        nc.sync.dma_start(out=bf2_t[:], in_=bf2[:].rearrange("l p t -> p l t"))
        bhw_t = sb.tile([128, 5, NT], F32)
        nc.sync.dma_start(out=bhw_t[:], in_=bhw[:].rearrange("l p t -> p l t"))
        bclsb_t = sb.tile([128, 3], F32)
        nc.sync.dma_start(out=bclsb_t[:], in_=bclsb[:])

        # ---------------- residual stream ----------------
        xt = sb.tile([128, NT, L], F32)
        nc.sync.dma_start(out=xt[:], in_=x0t[:].rearrange("(t p) i -> p t i", p=128))

        # ---------------- helpers ----------------
        def ln(idx, out_tile):
            """LayerNorm over features of xt -> out_tile[:, t, :]."""
            s_ps = ps.tile([1, L], F32, tag="st", bufs=2)
            sq_ps = ps.tile([1, L], F32, tag="st", bufs=2)
            for t in range(NT):
                x16 = sb.tile([128, L], F16, tag="x16", bufs=3)
                nc.scalar.copy(x16[:], xt[:, t, :])
                nc.tensor.matmul(s_ps[:], ones_col[:], x16[:],
                                 start=(t == 0), stop=(t == NT - 1))
            for t in range(NT):
                xsq = sb.tile([128, L], F16, tag="xsq", bufs=3)
                nc.scalar.square(xsq[:], xt[:, t, :])
                nc.tensor.matmul(sq_ps[:], ones_col[:], xsq[:],
                                 start=(t == 0), stop=(t == NT - 1))
            m = sb.tile([1, L], F32, tag="lnv", bufs=6)
            nc.vector.tensor_scalar_mul(m[:], s_ps[:], 1.0 / D)
            var = sb.tile([1, L], F32, tag="lnv", bufs=6)
            nc.vector.tensor_tensor(var[:], m[:], m[:], OP.mult)
            msq = sb.tile([1, L], F32, tag="lnv", bufs=6)
            nc.vector.tensor_scalar_mul(msq[:], sq_ps[:], 1.0 / D)
            nc.vector.tensor_tensor(var[:], msq[:], var[:], OP.subtract)
            nc.vector.tensor_scalar_add(var[:], var[:], EPS)
            rv = sb.tile([1, L], F32, tag="lnv", bufs=6)
            nc.vector.reciprocal(rv[:], var[:])
            s = sb.tile([1, L], F32, tag="lnv", bufs=6)
            nc.scalar.sqrt(s[:], rv[:])
            c = sb.tile([1, L], F32, tag="lnv", bufs=6)
            nc.vector.tensor_tensor(c[:], m[:], s[:], OP.mult)
            s_b = sb.tile([128, L], F32, tag="lnsb", bufs=2)
            nc.gpsimd.partition_broadcast(s_b[:], s[:], channels=128)
            c_b = sb.tile([128, L], F32, tag="lncb", bufs=2)
            nc.gpsimd.partition_broadcast(c_b[:], c[:], channels=128)
            for t in range(NT):
                z = sb.tile([128, L], F32, tag="lnz", bufs=2)
                nc.vector.tensor_tensor(z[:], xt[:, t, :], s_b[:], OP.mult)
                nc.vector.tensor_tensor(z[:], z[:], c_b[:], OP.subtract)
                nc.vector.tensor_scalar(
                    out_tile[:, t, :], z[:],
                    lnp[:, idx, 0, t, None], lnp[:, idx, 1, t, None],
                    OP.mult, OP.add)

        def wstream(src2d, r0, c0, rows=128, cols=512):
            w = sb.tile([128, cols], F16, tag="wt", bufs=6)
            nc.sync.dma_start(out=w[:, :], in_=src2d[r0:r0 + rows, c0:c0 + cols])
            return w

        def proj_fm(src3, l, rhs_tile, evac):
            """out^T[ot] = sum_kt W[kt,ot].T @ rhs[kt]; streams weights in
            [128,512] chunks, 4 psum accumulators at a time."""
            for h0 in (0, 4):
                psums = [ps.tile([128, L], F32, tag="pp", bufs=4,
                                 name=f"pj_{h0}_{o}") for o in range(4)]
                for kt in range(NT):
                    w = wstream(src3[l], kt * 128, h0 * 128)
                    for o in range(4):
                        nc.tensor.matmul(psums[o][:], w[:, o * 128:(o + 1) * 128],
                                         rhs_tile[:, kt, :],
                                         start=(kt == 0), stop=(kt == NT - 1))
                for o in range(4):
                    evac(h0 + o, psums[o])

        # ---------------- highway + fc ----------------
        def x_to_f16():
            x16f = sb.tile([128, NT, L], F16, tag="res", bufs=1, name="x16f")
            for t in range(NT):
                nc.scalar.copy(x16f[:, t, :], xt[:, t, :])
            return x16f

        if use_highway:
            for hl in range(2):
                x16f = x_to_f16()
                for qd in range(4):
                    ph = [ps.tile([128, L], F32, tag="pp", bufs=4,
                                  name=f"ph_{hl}_{qd}_{o}") for o in range(2)]
                    pt = [ps.tile([128, L], F32, tag="pp", bufs=4,
                                  name=f"pt_{hl}_{qd}_{o}") for o in range(2)]
                    for kt in range(NT):
                        wlq = wstream(whw[2 * hl], kt * 128, qd * 256, cols=256)
                        wgq = wstream(whw[2 * hl + 1], kt * 128, qd * 256, cols=256)
                        for o in range(2):
                            nc.tensor.matmul(ph[o][:], wlq[:, o * 128:(o + 1) * 128],
                                             x16f[:, kt, :],
                                             start=(kt == 0), stop=(kt == NT - 1))
                            nc.tensor.matmul(pt[o][:], wgq[:, o * 128:(o + 1) * 128],
                                             x16f[:, kt, :],
                                             start=(kt == 0), stop=(kt == NT - 1))
                    for o in range(2):
                        ot = qd * 2 + o
                        hx = sb.tile([128, L], F32, tag="hx", bufs=2)
                        nc.scalar.activation(hx[:], ph[o][:], AF.Gelu,
                                             bias=bhw_t[:, 2 * hl, ot, None])
                        tx = sb.tile([128, L], F32, tag="tx", bufs=2)
                        nc.scalar.activation(tx[:], pt[o][:], AF.Sigmoid,
                                             bias=bhw_t[:, 2 * hl + 1, ot, None])
                        dd = sb.tile([128, L], F32, tag="hwd", bufs=2)
                        nc.vector.tensor_tensor(dd[:], hx[:], xt[:, ot, :], OP.subtract)
                        nc.vector.tensor_tensor(dd[:], tx[:], dd[:], OP.mult)
                        nc.vector.tensor_tensor(xt[:, ot, :], xt[:, ot, :], dd[:], OP.add)
            # fc (replaces x)
            x16f = x_to_f16()

            def evac_fc(ot, p):
                nc.scalar.activation(xt[:, ot, :], p[:], AF.Identity,
                                     bias=bhw_t[:, 4, ot, None])
            proj_fm(whw, 4, x16f, evac_fc)

            for t in range(NT):
                pe = sb.tile([128, L], F32, tag="pe", bufs=2)
                nc.sync.dma_start(out=pe[:], in_=post[t * 128:(t + 1) * 128, :])
                nc.vector.tensor_tensor(xt[:, t, :], xt[:, t, :], pe[:], OP.add)
            ln(LN_POS, xt)
            for t in range(NT):
                se = sb.tile([128, L], F32, tag="pe", bufs=2)
                nc.sync.dma_start(out=se[:], in_=segt[t * 128:(t + 1) * 128, :])
                nc.vector.tensor_tensor(xt[:, t, :], xt[:, t, :], se[:], OP.add)
            ln(LN_SEG, xt)

        # ---------------- transformer layers ----------------
        for l in range(n_layers):
            res = sb.tile([128, NT, L], F16, tag="res", bufs=1, name=f"res_{l}")
            ln(LN_IN(l), res)

            # K projection -> bounce (feature-major [D, L] f16)
            kv_in_k = dram.tile([D, L], F16, tag="kvik", bufs=2)
            kv_out_k = dram.tile([2, D, L], F16, tag="kvok", bufs=2)

            def evac_k(ot, p, _kv=kv_in_k, _l=l):
                ke = sb.tile([128, L], F16, tag="kev", bufs=3)
                nc.scalar.activation(ke[:], p[:], AF.Identity,
                                     bias=bqko_t[:, _l, 1, ot, None])
                nc.sync.dma_start(out=_kv[ot * 128:(ot + 1) * 128, :], in_=ke[:])
            proj_fm(wk, l, res, evac_k)
            nc.gpsimd.collective_compute(
                "AllGather", OP.bypass, replica_groups=GROUPS,
                ins=[kv_in_k[:].opt()], outs=[kv_out_k[:].opt()])

            # V projection (token-major [L, D]) -> bounce
            kv_in_v = dram.tile([L, D], F16, tag="kviv", bufs=2)
            kv_out_v = dram.tile([2, L, D], F16, tag="kvov", bufs=2)
            bvr_l = sb.tile([1, D], F16, tag="bvr", bufs=2)
            nc.sync.dma_start(out=bvr_l[:], in_=bvr[l:l + 1, :])
            for dc in range(2):
                pv = [ps.tile([128, L], F32, tag="pp", bufs=4,
                              name=f"pv_{dc}_{jt}") for jt in range(LT)]
                for kt in range(NT):
                    w = wstream(wv[l], kt * 128, dc * 512)
                    for jt in range(LT):
                        nc.tensor.matmul(
                            pv[jt][:], res[:, kt, jt * 128:(jt + 1) * 128],
                            w[:, :], start=(kt == 0), stop=False)
                for jt in range(LT):
                    nc.tensor.matmul(pv[jt][:], ones_row[:],
                                     bvr_l[:, dc * 512:(dc + 1) * 512],
                                     start=False, stop=True)
                    ve = sb.tile([128, 512], F16, tag="vev", bufs=3)
                    nc.scalar.copy(ve[:], pv[jt][:])
                    nc.sync.dma_start(
                        out=kv_in_v[jt * 128:(jt + 1) * 128,
                                    dc * 512:(dc + 1) * 512],
                        in_=ve[:])
            nc.gpsimd.collective_compute(
                "AllGather", OP.bypass, replica_groups=GROUPS,
                ins=[kv_in_v[:].opt()], outs=[kv_out_v[:].opt()])

            # Q projection (scale 1/sqrt(dk) folded in; bias pre-scaled on host)
            q = sb.tile([128, NT, L], F16, tag="q", bufs=1, name=f"q_{l}")

            def evac_q(ot, p, _q=q, _l=l):
                nc.scalar.activation(_q[:, ot, :], p[:], AF.Identity,
                                     bias=bqko_t[:, _l, 0, ot, None], scale=0.125)
            proj_fm(wq, l, res, evac_q)

            # gather K^T (full j range) and V_aug
            Kg = sb.tile([128, NT, T], F16, tag="Kg", bufs=1, name=f"Kg_{l}")
            for r in range(2):
                nc.sync.dma_start(
                    out=Kg[:, :, r * L:(r + 1) * L],
                    in_=kv_out_k[r].rearrange("(t p) i -> p t i", p=128))
            Vaug = sb.tile([128, 2 * LT, H, DK + 1], F16, tag="Vaug", bufs=1,
                           name=f"Vaug_{l}")
            for r in range(2):
                for jt in range(LT):
                    nc.sync.dma_start(
                        out=Vaug[:, r * LT + jt, :, 0:DK],
                        in_=kv_out_v[r, jt * 128:(jt + 1) * 128, :]
                        .rearrange("p (h c) -> p h c", c=DK))
            nc.vector.memset(Vaug[:, :, :, DK:DK + 1], 1.0)

            # attention per head
            ctx = sb.tile([128, NT, L], F16, tag="ctx", bufs=1, name=f"ctx_{l}")
            for h in range(H):
                po = (h % 2) * 64
                dt = h // 2
                if h < NC_HEADS:
                    cbs = []
                    for cc in range(2):
                        cb = sb.tile([128, LT, L], F16, tag="cb", bufs=2,
                                     name=f"cb_{l}_{h}_{cc}")
                        nc.sync.dma_start(
                            out=cb[:],
                            in_=cb16[h, cc * 512:(cc + 1) * 512, :]
                            .rearrange("(jt p) i -> p jt i", p=128))
                        cbs.append(cb)
                P = sb.tile([128, 2 * LT, L], F16, tag="P", bufs=2,
                            name=f"P_{l}_{h}")
                for jt in range(2 * LT):
                    sp = ps.tile([128, L], F32, tag="pp", bufs=4, name="sp")
                    nc.tensor.matmul(
                        sp[:], Kg[po:po + 64, dt, jt * 128:(jt + 1) * 128],
                        q[po:po + 64, dt, :], start=True, stop=True)
                    if h < NC_HEADS:
                        st = sb.tile([128, L], F16, tag="sexp", bufs=3)
                        nc.vector.tensor_tensor(st[:], sp[:],
                                                cbs[jt // LT][:, jt % LT, :], OP.add)
                        nc.scalar.activation(P[:, jt, :], st[:], AF.Exp)
                    else:
                        nc.scalar.activation(P[:, jt, :], sp[:], AF.Exp)
                cp = ps.tile([DK + 1, L], F32, tag="pc", bufs=2, name="cp")
                for jt in range(2 * LT):
                    nc.tensor.matmul(cp[:], Vaug[:, jt, h, :], P[:, jt, :],
                                     start=(jt == 0), stop=(jt == 2 * LT - 1))
                rs = sb.tile([1, L], F32, tag="rs", bufs=2)
                nc.vector.reciprocal(rs[:], cp[DK:DK + 1, :])
                rb = sb.tile([64, L], F32, tag="rb", bufs=2)
                nc.gpsimd.partition_broadcast(rb[:], rs[:], channels=64)
                nc.vector.tensor_tensor(ctx[po:po + 64, dt, :], cp[0:DK, :],
                                        rb[:], OP.mult)

            # output projection + residual
            def evac_o(ot, p, _l=l):
                nc.vector.scalar_tensor_tensor(
                    xt[:, ot, :], p[:], bqko_t[:, _l, 2, ot, None],
                    xt[:, ot, :], OP.add, OP.add)
            proj_fm(wo, l, ctx, evac_o)

            # FFN
            res2 = sb.tile([128, NT, L], F16, tag="res", bufs=1, name=f"res2_{l}")
            ln(LN_OUT(l), res2)
            h1 = sb.tile([128, DFF // 128, L], F16, tag="h1", bufs=1,
                         name=f"h1_{l}")
            for g4 in range(8):
                pf = [ps.tile([128, L], F32, tag="pp", bufs=4,
                              name=f"pf_{g4}_{o}") for o in range(4)]
                for kt in range(NT):
                    w = wstream(wf1[l], kt * 128, g4 * 512)
                    for o in range(4):
                        nc.tensor.matmul(pf[o][:], w[:, o * 128:(o + 1) * 128],
                                         res2[:, kt, :],
                                         start=(kt == 0), stop=(kt == NT - 1))
                for o in range(4):
                    ft = g4 * 4 + o
                    nc.scalar.activation(h1[:, ft, :], pf[o][:], AF.Gelu,
                                         bias=bf1_t[:, l, ft, None])
            for half in range(2):
                pres = [ps.tile([128, L], F32, tag="pp", bufs=4,
                                name=f"pf2_{half}_{o}") for o in range(4)]
                for kt in range(DFF // 128):
                    w = wstream(wf2[l], kt * 128, half * 512)
                    for o in range(4):
                        nc.tensor.matmul(pres[o][:], w[:, o * 128:(o + 1) * 128],
                                         h1[:, kt, :],
                                         start=(kt == 0), stop=(kt == DFF // 128 - 1))
                for o in range(4):
                    ot = half * 4 + o
                    nc.vector.scalar_tensor_tensor(
                        xt[:, ot, :], pres[o][:], bf2_t[:, l, ot, None],
                        xt[:, ot, :], OP.add, OP.add)

        # ---------------- final LN + classifier ----------------
        resf = sb.tile([128, NT, L], F16, tag="res", bufs=1, name="resf")
        ln(LN_FINAL, resf)
        wc = sb.tile([128, NT, 3], F16)
        nc.sync.dma_start(out=wc[:], in_=wcls[:].rearrange("(t p) c -> p t c", p=128))
        osb = sb.tile([128, LT, 3], F32)
        for it in range(LT):
            p = ps.tile([128, 3], F32, tag="pp", bufs=4, name="pcls")
            for kt in range(NT):
                nc.tensor.matmul(p[:], resf[:, kt, it * 128:(it + 1) * 128],
                                 wc[:, kt, :], start=(kt == 0), stop=(kt == NT - 1))
            nc.vector.tensor_tensor(osb[:, it, :], p[:], bclsb_t[:], OP.add)
            nc.sync.dma_start(out=out[it * 128:(it + 1) * 128, :], in_=osb[:, it, :])

        es.close()
    return nc


# =====================  host side  =====================

def _prep_shared(params):
    f16 = np.float16
    f32 = np.float32

    def A(x):
        return np.asarray(x)

    lay = params["layers"]
    hw = params["highway"]
    d = {}
    d["whw"] = np.ascontiguousarray(np.stack(
        [A(hw["lin"][0]["w"]), A(hw["gate"][0]["w"]),
         A(hw["lin"][1]["w"]), A(hw["gate"][1]["w"]), A(hw["fc"]["w"])]
    ).astype(f16))
    d["bhw"] = np.ascontiguousarray(np.stack(
        [A(hw["lin"][0]["b"]), A(hw["gate"][0]["b"]),
         A(hw["lin"][1]["b"]), A(hw["gate"][1]["b"]), A(hw["fc"]["b"])]
    ).astype(f32).reshape(5, NT, 128).transpose(0, 2, 1))
    for nm in ["q", "k", "v", "o"]:
        d["w" + nm] = np.ascontiguousarray(
            np.stack([A(lp[nm]["w"]) for lp in lay]).astype(f16))
    d["wf1"] = np.ascontiguousarray(
        np.stack([A(lp["ff1"]["w"]) for lp in lay]).astype(f16))
    d["wf2"] = np.ascontiguousarray(
        np.stack([A(lp["ff2"]["w"]) for lp in lay]).astype(f16))

    def pack_bias(b):  # [1024] -> [128, 8]
        return A(b).astype(f32).reshape(NT, 128).T

    d["bqko"] = np.ascontiguousarray(np.stack([
        np.stack([pack_bias(A(lp["q"]["b"]) * 0.125),
                  pack_bias(lp["k"]["b"]),
                  pack_bias(lp["o"]["b"])]) for lp in lay]))
    d["bvr"] = np.ascontiguousarray(
        np.stack([A(lp["v"]["b"]) for lp in lay]).astype(f16))
    d["bf1"] = np.ascontiguousarray(np.stack(
        [A(lp["ff1"]["b"]).astype(f32).reshape(DFF // 128, 128).T for lp in lay]))
    d["bf2"] = np.ascontiguousarray(np.stack(
        [pack_bias(lp["ff2"]["b"]) for lp in lay]))

    lns = [params["pos_norm"], params["seg_norm"]]
    for lp in lay:
        lns += [lp["norm_in"], lp["norm_out"]]
    lns.append(params["final_norm"])
    d["ln_wb"] = np.ascontiguousarray(np.stack(
        [np.stack([pack_bias(p["w"]), pack_bias(p["b"])]) for p in lns]))
    d["wcls"] = np.ascontiguousarray(A(params["cls"]["w"]).astype(f16))
    d["bclsb"] = np.ascontiguousarray(
        np.broadcast_to(A(params["cls"]["b"]).astype(f32), (128, 3)).copy())
    return d


def make_in_maps(qa, segment_ids, mask, concept, params):
    qa = np.asarray(qa, dtype=np.float32)
    segment_ids = np.asarray(segment_ids)
    mask = np.asarray(mask)
    concept = np.asarray(concept, dtype=np.float32)
    assert np.all(mask == 1), "kernel assumes all-ones mask (per input_specs)"

    shared = _prep_shared(params)
    pos = np.asarray(params["pos_emb"])[:T].astype(np.float32)      # [T, D]
    seg = np.asarray(params["seg_emb"]).astype(np.float32)          # [4, D]

    in_maps = []
    for core in range(8):
        b, s = divmod(core, 2)
        sl = slice(s * L, (s + 1) * L)
        m = dict(shared)
        m["x0t"] = np.ascontiguousarray(qa[b, sl, :].T)
        m["post"] = np.ascontiguousarray(pos[sl, :].T)
        m["segt"] = np.ascontiguousarray(seg[segment_ids[b, sl]].T)
        m["cb16"] = np.ascontiguousarray(
            (LAMBD * concept[b, sl, :, :NC_HEADS]).transpose(2, 1, 0)
            .astype(np.float16))
        in_maps.append(m)
    return in_maps


_cached = {}


def kernel(qa, segment_ids, mask, concept, params):
    in_maps = make_in_maps(qa, segment_ids, mask, concept, params)
    if "nc" not in _cached:
        nc = build()
        if not nc.is_finalized():
            nc.finalize()
        _cached["nc"] = nc
    res = run_bass_kernel_spmd(_cached["nc"], in_maps, core_ids=list(range(8)))
    out = np.zeros((B, T, 3), np.float32)
    for core in range(8):
        b, s = divmod(core, 2)
        out[b, s * L:(s + 1) * L, :] = res.results[core]["out"]
    return out
